# revision 12
# baseline (speedup 1.0000x reference)
"""Binarized DenseNet (nn_DenseNet_5841155522939) Trainium2 Bass kernel.

Strategy (data-parallel over batch, 8 cores x 32 samples):
  - Activations are {0,1}-encoded (a = (s+1)/2, pads stored as 0.5 so padded
    taps contribute 0 to the +-1-world conv); weights are +-1 fp8e4. The
    +-1-world conv value is c = 2c' - W1 (W1 = per-out-channel weight sum),
    absorbed into per-channel thresholds host-side, all decisions bit-exact.
  - conv2-5 use fp8 DoubleRow matmuls (0.5 cycles/row): per 512-pixel chunk,
    3 matmuls x in {0,1,2} cover all 9 taps: the k-tile pair (free-dim stride
    WP) gives tap rows -1/0, the +2WP-shifted partition half (canvas rows
    64-127) gives row +1, and the 4th slot is zero-weighted.
  - DoubleRow outputs must start at PSUM partition 0, so conv outputs land in
    [64, 1024] PSUM tiles; the sign/threshold step writes the {0,1} canvas
    DIRECTLY (no staging DMAs), spread across all three elementwise engines:
    Act (saturated sigmoid at 2^80 scale => exact {0,1}), DVE and GpSimd
    (tensor_scalar mult+is_ge).
  - conv6 (stride 2) reads a dedicated pitch-128 canvas (the k-tile stride
    must be a 64-multiple when the moving AP's last dim is strided): 3
    DoubleRow matmuls per sample-half.
  - Layer 1 (cin=1, stride 2, f32 input) streams an exact 3-way bf16 split
    of x through a single K=27 matmul (9 taps x 3 splits), built via a
    parity-split DRAM staging roundtrip.
  - FC uses 256 DoubleRow matmuls (k-tiles = feature pairs p, p+256), with
    the {0,1}->+-1 correction folded into a final scale-2/bias op.
"""

import numpy as np
from contextlib import ExitStack

import concourse.bacc as bacc
import concourse.bass as bass
import concourse.tile as tile
from concourse import mybir
from concourse.bass_utils import run_bass_kernel_spmd

FP8 = mybir.dt.float8e4
BF16 = mybir.dt.bfloat16
F32 = mybir.dt.float32
NP_FP8 = mybir.dt.np(FP8)
NP_BF16 = mybir.dt.np(BF16)
DR = mybir.MatmulPerfMode.DoubleRow
ALU = mybir.AluOpType
SIG = mybir.ActivationFunctionType.Sigmoid

B = 256
NCORES = 8
BPC = B // NCORES          # 32 samples per core
GB = 4                     # samples per group
G = BPC // GB              # 8 groups
NCH = 64
WP = 66                    # padded canvas row stride (64 + 2)
S1 = WP * WP               # 4356 canvas cells per sample
S = GB * S1                # canvas cells per group
W6P = 128                  # conv6 canvas row pitch
C6 = 66 * W6P              # conv6 canvas cells per sample
EPS = np.float32(1e-5)
KSAT = np.float32(2.0 ** 80)   # sigmoid saturation scale (exact power of 2)


def _thresholds(g, b, m, v, cmax=600):
    """Per-channel (scale, bias) s.t. Sign(scale*c + bias) == reference
    sign((c-m)*g*rsqrt(v+eps)+b) for every integer c in [-cmax, cmax]."""
    inv = (np.float32(1.0) / np.sqrt((v + EPS).astype(np.float32))).astype(np.float32)
    s = (g * inv).astype(np.float32)
    C = np.arange(-cmax, cmax + 1, dtype=np.float32)
    P = len(g)
    scale = np.zeros(P, np.float32)
    bias = np.zeros(P, np.float32)
    for c in range(P):
        vals = ((C - m[c]) * s[c] + b[c]).astype(np.float32)
        sg = np.sign(vals)
        if np.any(sg == 0.0):
            raise RuntimeError(f"exact-zero BN output, channel {c}")
        if np.all(sg == sg[0]):
            scale[c] = 0.0
            bias[c] = sg[0]
            continue
        d = np.diff(sg)
        idx = np.nonzero(d)[0]
        if len(idx) != 1:
            raise RuntimeError(f"non-monotone BN sign, channel {c}")
        T = C[idx[0] + 1]
        if sg[0] < 0:
            scale[c] = 1.0
            bias[c] = np.float32(-(T - 0.5))
        else:
            scale[c] = -1.0
            bias[c] = np.float32(T - 0.5)
    return scale, bias


def _sap(t, prow, pcount, off, dims):
    """AP into tile/tensor ap `t` ([:] view): partition rows [prow, prow+pcount),
    free offset `off` elements, free dims `dims` = [[step, count], ...]."""
    ps = t.ap[0][0]
    return bass.AP(tensor=t.tensor, offset=t.offset + prow * ps + off,
                   ap=[[ps, pcount]] + dims)


def _memset_pads(nc, canvas):
    """Set the pad cells of copy0 rows 0-63 to the {0,1}-encoding of a zero
    activation (0.5). Everything else is written before being read."""
    t = canvas[:]
    nc.gpsimd.memset(_sap(t, 0, 64, 0, [[S1, GB], [65 * WP, 2], [1, WP]]), 0.5)
    nc.gpsimd.memset(_sap(t, 0, 64, 0, [[S1, GB], [WP, WP], [65, 2]]), 0.5)
    # unmaintained 132-cell tail of each 2-sample block on the shifted-copy
    # partitions: only ever read by the zero-weighted k-tile1 hi slot, but
    # must not be NaN (NaN * 0 = NaN poisons PSUM)
    nc.gpsimd.memset(_sap(t, 64, 64, S1 - 132, [[S1, GB], [1, 132]]), 0.5)


def _build_nc():
    nc = bacc.Bacc("TRN2", target_bir_lowering=False, debug=False,
                   num_devices=NCORES)
    d_x = nc.dram_tensor("xs", [BPC, 1, 128, 128], F32, kind="ExternalInput")
    d_w1 = nc.dram_tensor("w1s", [27, NCH], BF16, kind="ExternalInput")
    d_wc = nc.dram_tensor("wc", [128, 4, 3, 2, NCH], FP8, kind="ExternalInput")
    d_w6 = nc.dram_tensor("w6d", [128, 3, 2, NCH], FP8, kind="ExternalInput")
    d_sb = nc.dram_tensor("sb", [64, 4, 6], F32, kind="ExternalInput")
    d_wfc = nc.dram_tensor("wfc_l", [128, 64, 96], FP8, kind="ExternalInput")
    d_bfc = nc.dram_tensor("bfc_t", [12, 1], F32, kind="ExternalInput")
    d_out = nc.dram_tensor("out", [BPC, 12], F32, kind="ExternalOutput")

    with tile.TileContext(nc) as tc, ExitStack() as ctx:
        constp = ctx.enter_context(tc.tile_pool(name="const", bufs=1))
        canvasp = ctx.enter_context(tc.tile_pool(name="canvas", bufs=3))
        canvas6p = ctx.enter_context(tc.tile_pool(name="canvas6", bufs=1))
        x27p = ctx.enter_context(tc.tile_pool(name="x27", bufs=2))
        frontp = ctx.enter_context(tc.tile_pool(name="front", bufs=1))
        psump = ctx.enter_context(tc.tile_pool(name="psum", bufs=4, space="PSUM"))
        dramp = ctx.enter_context(tc.tile_pool(name="dram", bufs=2, space="DRAM"))

        # ---- constants
        w1 = constp.tile([27, NCH], BF16)
        wc = constp.tile([128, 4, 3, 2, NCH], FP8)
        w6 = constp.tile([128, 3, 2, NCH], FP8)
        # sb rows: 0 = s1, 1 = s2, 2 = KSAT*s1, 3 = -KSAT*s2
        sb = constp.tile([64, 4, 6], F32)
        wfc = constp.tile([128, 64, 96], FP8)
        bfc = constp.tile([12, 1], F32)
        act6 = constp.tile([128, BPC * 1024], FP8)
        first_canvases = []
        first_canvas6 = []
        first_x27 = []

        # post-proc engine rotation (GPSIMD cannot read PSUM): Act 8 : DVE 7
        ROT = "ADADADADADADADA"
        rot_state = [0]

        def post(ps_tile, out_ap, li):
            """{0,1} threshold step: out = [s1*c >= s2], one elementwise op
            on the next engine in the rotation."""
            e = ROT[rot_state[0] % len(ROT)]
            rot_state[0] += 1
            if e == "A":
                nc.scalar.activation(out_ap, ps_tile[:], SIG,
                                     bias=sb[:, 3, li:li + 1],
                                     scale=sb[:, 2, li:li + 1])
            else:
                nc.vector.tensor_scalar(out_ap, ps_tile[:],
                                        sb[:, 0, li:li + 1], sb[:, 1, li:li + 1],
                                        ALU.mult, ALU.is_ge)

        def emit_front(g):
            """Layer-1 input pipeline for group g: load x, 3-way bf16 split,
            parity rearrange, DRAM staging, X27 gathers. Returns X27 halves."""
            X = frontp.tile([128, GB * 128], F32, tag="X")
            nc.sync.dma_start(out=X[:], in_=bass.AP(
                tensor=d_x[:].tensor, offset=g * GB * 16384,
                ap=[[128, 128], [16384, GB], [1, 128]]))
            s0u = frontp.tile([128, GB * 128], BF16, tag="s0u")
            s1u = frontp.tile([128, GB * 128], BF16, tag="s1u")
            s2u = frontp.tile([128, GB * 128], BF16, tag="s2u")
            r1 = frontp.tile([128, GB * 128], F32, tag="r1")
            r2 = frontp.tile([128, GB * 128], F32, tag="r2")
            nc.gpsimd.tensor_copy(s0u[:], X[:])
            nc.gpsimd.tensor_sub(r1[:], X[:], s0u[:])
            nc.gpsimd.tensor_copy(s1u[:], r1[:])
            nc.gpsimd.tensor_sub(r2[:], r1[:], s1u[:])
            nc.gpsimd.tensor_copy(s2u[:], r2[:])
            SP = frontp.tile([128, 3, GB, 2, 64], BF16, tag="SP")
            for si, st in enumerate((s0u, s1u, s2u)):
                for px in range(2):
                    nc.gpsimd.tensor_copy(
                        SP[:, si, :, px, :],
                        bass.AP(tensor=st[:].tensor, offset=st[:].offset + px,
                                ap=[[GB * 128, 128], [128, GB], [2, 64]]))
            FRO = 3 * GB * 2 * 64
            DS = dramp.tile([2 * 64 * FRO], BF16)
            dsap = DS[:]
            for py in range(2):
                src = bass.AP(tensor=SP[:].tensor,
                              offset=SP[:].offset + py * FRO,
                              ap=[[2 * FRO, 64], [1, FRO]])
                dst = bass.AP(tensor=dsap.tensor,
                              offset=dsap.offset + py * (3 * GB * 2 * 4096),
                              ap=[[64, 64], [4096, 24], [1, 64]])
                nc.sync.dma_start(out=dst, in_=src)
            halves = []
            for hb in range(2):
                X27 = x27p.tile([32, 2 * 4096], BF16, tag="x27")
                if len(first_x27) < 2:
                    first_x27.append(X27)
                    nc.gpsimd.memset(X27[:], 0.0)
                for t9 in range(9):
                    dy, dx = t9 // 3, t9 % 3
                    pyy, rh = (dy - 1) % 2, (dy - 1 - (dy - 1) % 2) // 2
                    pxx, rw = (dx - 1) % 2, (dx - 1 - (dx - 1) % 2) // 2
                    h0, w0 = -rh, -rw
                    cnt_h, cnt_w = 64 - h0, 64 - w0
                    for bb in range(2):
                        soff = (dsap.offset + pyy * (3 * GB * 2 * 4096)
                                + pxx * 4096 + (h0 + rh) * 64 + (w0 + rw)
                                + (hb * 2 + bb) * 2 * 4096)
                        src = bass.AP(tensor=dsap.tensor, offset=soff,
                                      ap=[[GB * 2 * 4096, 3],
                                          [64, cnt_h], [1, cnt_w]])
                        dst = _sap(X27[:], 3 * t9, 3,
                                   bb * 4096 + h0 * 64 + w0,
                                   [[64, cnt_h], [1, cnt_w]])
                        nc.sync.dma_start(out=dst, in_=src)
                halves.append(X27)
            return halves

        front = emit_front(0)
        nc.sync.dma_start(out=w1[:], in_=d_w1[:])
        nc.sync.dma_start(out=wc[:], in_=d_wc[:])
        nc.sync.dma_start(out=w6[:], in_=d_w6[:])
        nc.sync.dma_start(out=sb[:], in_=d_sb[:])
        nc.sync.dma_start(out=wfc[:], in_=d_wfc[:])
        nc.sync.dma_start(out=bfc[:], in_=d_bfc[:])
        for g in range(G):
            # ================= conv1 matmuls -> L1 canvas =================
            x27_halves = front
            cur = canvasp.tile([128, S], FP8, tag="canvas")
            if len(first_canvases) < 3:
                first_canvases.append(cur)
                _memset_pads(nc, cur)
            for hb in range(2):
                X27 = x27_halves[hb]
                for b2 in range(2):
                    b = 2 * hb + b2
                    for q in range(4):
                        pst = psump.tile([64, 1024], F32, tag="pst")
                        for half in range(2):
                            h0 = 16 * q + 8 * half
                            rhs = _sap(X27[:], 0, 27, b2 * 4096 + h0 * 64,
                                       [[64, 8], [1, 64]])
                            nc.tensor.matmul(
                                pst[:, 512 * half:512 * half + 512],
                                lhsT=w1[:], rhs=rhs, start=True, stop=True)
                        post(pst, _sap(cur[:], 0, 64,
                                       b * S1 + (16 * q + 1) * WP + 1,
                                       [[WP, 16], [1, 64]]), 0)
                    eng = nc.scalar if b % 2 == 0 else nc.sync
                    eng.dma_start(
                        out=_sap(cur[:], 64, 64, b * S1, [[1, S1 - 132]]),
                        in_=_sap(cur[:], 0, 64, b * S1 + 132, [[1, S1 - 132]]))

            # ================= conv2..conv5 (DoubleRow) =================
            for li in range(4):
                if li == 0 and g + 1 < G:
                    front = emit_front(g + 1)
                last = li == 3
                if last:
                    nxt6 = canvas6p.tile([128, GB * C6], FP8, tag="canvas6")
                    if len(first_canvas6) < 1:
                        first_canvas6.append(nxt6)
                        t6 = nxt6[:]
                        # top pad (slot 0) + bottom pad (slot 65) + left pad
                        # col, {0,1}-zero = 0.5
                        nc.gpsimd.memset(
                            _sap(t6, 0, 64, 0,
                                 [[C6, GB], [65 * W6P, 2], [1, W6P]]), 0.5)
                        nc.gpsimd.memset(
                            _sap(t6, 0, 64, 0, [[C6, GB], [W6P, 66], [1, 1]]), 0.5)
                        nc.gpsimd.memset(
                            _sap(t6, 64, 64, C6 - 2 * W6P,
                                 [[C6, GB], [1, 2 * W6P]]), 0.5)
                else:
                    nxt = canvasp.tile([128, S], FP8, tag="canvas")
                    if len(first_canvases) < 3:
                        first_canvases.append(nxt)
                        _memset_pads(nc, nxt)
                for b in range(GB):
                    for t in range(4):
                        pst = psump.tile([64, 1024], F32, tag="pst")
                        for ci in range(2):
                            c = 2 * t + ci
                            for x in range(3):
                                # k-tile0: tap row -1, k-tile1: row 0;
                                # +2WP partition half: row +1; 4th slot zeroed
                                rhs = _sap(cur[:], 0, 128,
                                           b * S1 + 8 * c * WP + x,
                                           [[WP, 2], [WP, 8], [1, 64]])
                                nc.tensor.matmul(
                                    pst[:, 512 * ci:512 * ci + 512],
                                    lhsT=wc[:, li, x, :, :], rhs=rhs,
                                    start=(x == 0), stop=(x == 2),
                                    perf_mode=DR)
                        if last:
                            out_ap = _sap(nxt6[:], 0, 64,
                                          b * C6 + (16 * t + 1) * W6P + 1,
                                          [[W6P, 16], [1, 64]])
                        else:
                            out_ap = _sap(nxt[:], 0, 64,
                                          b * S1 + (16 * t + 1) * WP + 1,
                                          [[WP, 16], [1, 64]])
                        post(pst, out_ap, li + 1)
                    eng = nc.scalar if b % 2 == 0 else nc.sync
                    if last:
                        t6 = nxt6[:]
                        eng.dma_start(
                            out=_sap(t6, 64, 64, b * C6, [[1, C6 - 2 * W6P]]),
                            in_=_sap(t6, 0, 64, b * C6 + 2 * W6P,
                                     [[1, C6 - 2 * W6P]]))
                    else:
                        eng.dma_start(
                            out=_sap(nxt[:], 64, 64, b * S1, [[1, S1 - 132]]),
                            in_=_sap(nxt[:], 0, 64, b * S1 + 132,
                                     [[1, S1 - 132]]))
                if not last:
                    cur = nxt

            # ================= conv6 (stride 2, DoubleRow) -> act6 =========
            for b in range(GB):
                pst = psump.tile([64, 1024], F32, tag="pst")
                for half in range(2):
                    h0 = 16 * half
                    for x in range(3):
                        rhs = _sap(nxt6[:], 0, 128,
                                   b * C6 + 2 * h0 * W6P + x,
                                   [[W6P, 2], [2 * W6P, 16], [2, 32]])
                        nc.tensor.matmul(
                            pst[:, 512 * half:512 * half + 512],
                            lhsT=w6[:, x, :, :], rhs=rhs,
                            start=(x == 0), stop=(x == 2), perf_mode=DR)
                post(pst, _sap(act6[:], 0, 64, (g * GB + b) * 1024,
                               [[1, 1024]]), 5)
            # upper fc operand half for this group (act6 shifted +512/sample)
            nc.sync.dma_start(
                out=_sap(act6[:], 64, 64, g * GB * 1024, [[1024, GB], [1, 512]]),
                in_=_sap(act6[:], 0, 64, g * GB * 1024 + 512, [[1024, GB], [1, 512]]))

        # ================= fc (plain fp8, K=128 feature chunks) ============
        psf = psump.tile([64, 1024], F32, tag="pst")
        for p in range(512):
            rhs = _sap(act6[:], 0, 128, p, [[1024, BPC]])
            nc.tensor.matmul(psf[0:12, 0:BPC],
                             lhsT=wfc[:, p // 8, 12 * (p % 8):12 * (p % 8) + 12],
                             rhs=rhs, start=(p == 0), stop=(p == 511))
        accf = constp.tile([12, BPC], F32)
        # {0,1} -> +-1 correction: out = 2*psf + (bfc - rowsum(sign(wfc)))
        nc.vector.tensor_scalar(accf[:], psf[0:12, 0:BPC], 2.0, bfc[:],
                                ALU.mult, ALU.add)
        nc.sync.dma_start(
            out=bass.AP(tensor=d_out[:].tensor, offset=0,
                        ap=[[1, 12], [12, BPC]]),
            in_=accf[:])

    nc.compile()
    return nc


_NC_CACHE = {}


def _prep_const_inputs(inputs):
    out = {}
    # layer-1 weights: [27 = 3*t + s, cout] bf16 (same tap weight per split)
    w1b = np.sign(np.asarray(inputs["w1"], np.float32))  # [64, 1, 3, 3]
    w1s = np.zeros((27, NCH), NP_BF16)
    for t9 in range(9):
        dy, dx = t9 // 3, t9 % 3
        for s3 in range(3):
            w1s[3 * t9 + s3, :] = w1b[:, 0, dy, dx].astype(NP_BF16)
    out["w1s"] = w1s
    # conv2-5 DoubleRow weights [128, li, x, ktile, cout] + W1 sums
    wc = np.zeros((128, 4, 3, 2, NCH), NP_FP8)
    w1sums = {}
    for li in range(4):
        w = np.sign(np.asarray(inputs[f"w{li + 2}"], np.float32))  # [O,I,3,3]
        w1sums[li + 1] = w.sum(axis=(1, 2, 3)).astype(np.float32)
        for x in range(3):
            wc[0:64, li, x, 0, :] = w[:, :, 0, x].T.astype(NP_FP8)
            wc[0:64, li, x, 1, :] = w[:, :, 1, x].T.astype(NP_FP8)
            wc[64:128, li, x, 0, :] = w[:, :, 2, x].T.astype(NP_FP8)
    out["wc"] = wc
    # conv6 DoubleRow weights [128, x, ktile, cout]
    w6r = np.sign(np.asarray(inputs["w6"], np.float32))
    w1sums[5] = w6r.sum(axis=(1, 2, 3)).astype(np.float32)
    w6d = np.zeros((128, 3, 2, NCH), NP_FP8)
    for x in range(3):
        w6d[0:64, x, 0, :] = w6r[:, :, 0, x].T.astype(NP_FP8)
        w6d[0:64, x, 1, :] = w6r[:, :, 1, x].T.astype(NP_FP8)
        w6d[64:128, x, 0, :] = w6r[:, :, 2, x].T.astype(NP_FP8)
    out["w6d"] = w6d
    # thresholds [64, 4, 6]: rows (s1, s2, K*s1, -K*s2) per layer.
    # Decision for layer L: [s1*c' >= s2] where c' is the {0,1}-conv value.
    sb = np.zeros((64, 4, 6), np.float32)
    g1, b1, m1, v1 = (np.asarray(inputs[k], np.float32) for k in
                      ("g1", "b1", "m1", "v1"))
    inv = (np.float32(1.0) / np.sqrt((v1 + EPS).astype(np.float32))).astype(np.float32)
    s1c = (g1 * inv).astype(np.float32)
    sb[:, 0, 0] = s1c
    sb[:, 1, 0] = (m1 * s1c - b1).astype(np.float32)
    for li in range(1, 6):
        g_, b_, m_, v_ = (np.asarray(inputs[f"{k}{li + 1}"], np.float32)
                          for k in ("g", "b", "m", "v"))
        sc, bi = _thresholds(g_, b_, m_, v_)
        # +-1-world: [sc*c + bi >= 0], c = 2c' - W1
        sb[:, 0, li] = (2.0 * sc).astype(np.float32)
        sb[:, 1, li] = (sc * w1sums[li] - bi).astype(np.float32)
    sb[:, 2, :] = KSAT * sb[:, 0, :]
    sb[:, 3, :] = -KSAT * sb[:, 1, :]
    out["sb"] = sb
    # fc: lhsT [128, chunk k, m = j*12 + cls] (plain matmuls)
    wfcs = np.sign(np.asarray(inputs["wfc"], np.float32)).reshape(12, 64, 1024)
    wl = np.zeros((128, 64, 96), NP_FP8)
    for k in range(64):
        for j in range(8):
            p = 8 * k + j
            wl[0:64, k, 12 * j:12 * j + 12] = wfcs[:, :, p].T.astype(NP_FP8)
            wl[64:128, k, 12 * j:12 * j + 12] = wfcs[:, :, p + 512].T.astype(NP_FP8)
    out["wfc_l"] = wl
    bfc2 = (np.asarray(inputs["bfc"], np.float32)
            - wfcs.sum(axis=(1, 2)).astype(np.float32))
    out["bfc_t"] = bfc2.reshape(12, 1).astype(np.float32)
    return out


def kernel(**inputs):
    if "nc" not in _NC_CACHE:
        _NC_CACHE["nc"] = _build_nc()
    nc = _NC_CACHE["nc"]
    const = _prep_const_inputs(inputs)
    x = np.asarray(inputs["x"], np.float32)
    in_maps = []
    for c in range(NCORES):
        m = dict(const)
        m["xs"] = np.ascontiguousarray(x[c * BPC:(c + 1) * BPC])
        in_maps.append(m)
    res = run_bass_kernel_spmd(nc, in_maps, core_ids=list(range(NCORES)))
    return np.concatenate([r["out"] for r in res.results], axis=0)
